# revision 44
# baseline (speedup 1.0000x reference)
"""Multi-head self-attention (B=4, S=2048, D=1024, H=16) on 8 trn2 NeuronCores.

Sharding: core c -> batch b = c//2, head-group g = c%2 (8 heads, 512 of the
1024 output/QKV columns). Each core computes Q/K/V projections for its slice
and full attention for its 8 heads. Host does layout prep (x transpose + bf16
cast, W column slices) and the final gather/transpose - no collectives needed.

All matmuls in bf16 (psum accumulation f32): full PE rate, half the weight-load
time and DMA of f32r, and lower PE power draw (the f32r version tripped the HW
utilization throttle to ~54% duty).

Single fused pipeline per core (projections threaded into attention; the
ACT engine's exp stream is the binding resource at ~1.11us per k-block, so
everything else hides under it):
  prefix: K for head-pair 0, Q chunk 0, first two V groups. Inputs arrive
          via all three hwdge DMA queues, swizzled host-side so every
          transfer is contiguous per partition.
  per head-pair hp, per q-chunk(512): software-pipelined over 16 k-blocks:
      scoresT[k,q] psum[128,1024] <- KT-tile.T @ QT-chunk (2 heads, one bank
        each, tile_position rows 0/64 - the pair executes concurrently);
      one ACT exp over both banks -> ex bf16 [128,1024];
      pv[65,512] psum += Vx-tile.T @ ex-half (row 64 = denominator via a
        ones column in Vx), issued one k-block behind the scores;
      threaded into the iterations: the remaining V groups (during q-block
        0 of head-pair 0) and this/next head-pair's K/Q projection groups
        (one per ~6 iterations), so the PE fills its exp-stall slack.
    normalize: pv is copied out of PSUM immediately (frees the bank), then
      out = pv[0:64] * partition_broadcast(recip(pv[64])), per-head DMA out.
  output: outT[128,4,2048] f32 per core (partition-major); host transposes.
"""
import numpy as np
import ml_dtypes

import concourse.bacc as bacc
import concourse.mybir as mybir
import concourse.tile as tile
from concourse.bass_utils import run_bass_kernel_spmd

B, S, D, H = 4, 2048, 1024, 16
DH = D // H            # 64
NCORES = 8
HLOC = H // 2          # 8 heads per core
DLOC = HLOC * DH       # 512 output cols per core
NM = DLOC // 128       # 4 head-pair blocks
F32 = mybir.dt.float32
BF16 = mybir.dt.bfloat16
EXPF = mybir.ActivationFunctionType.Exp

SC = 512               # s-chunk for projections
QC = 512               # q-chunk in attention
NKB = S // 128         # 16 k-blocks
NDT = D // 128         # 8 contraction tiles for QKV


def _build():
    nc = bacc.Bacc("TRN2", target_bir_lowering=False, debug=False, num_devices=NCORES)
    # Host pre-swizzles everything partition-major so every DMA moves one
    # fully contiguous block per partition (32KB descriptors, not 1KB).
    xT = nc.dram_tensor("xT", [128, S // SC, NDT, SC], BF16,
                        kind="ExternalInput").ap()
    Wq = nc.dram_tensor("Wq", [128, NDT, DLOC], BF16, kind="ExternalInput").ap()
    Wk = nc.dram_tensor("Wk", [128, NDT, DLOC], BF16, kind="ExternalInput").ap()
    Wv = nc.dram_tensor("Wv", [128, NDT, DLOC], BF16, kind="ExternalInput").ap()
    out = nc.dram_tensor("outT", [128, NM, S], F32, kind="ExternalOutput").ap()

    out_t = out                                           # [128, 4, 2048]

    with tile.TileContext(nc) as tc:
        with tc.tile_pool(name="persist", bufs=1) as keep, \
             tc.tile_pool(name="p2e", bufs=4) as p2e, \
             tc.tile_pool(name="p2n", bufs=2) as p2n, \
             tc.tile_pool(name="p1ps", bufs=2, space="PSUM") as p1ps, \
             tc.tile_pool(name="ps_s", bufs=2, space="PSUM") as ps_s, \
             tc.tile_pool(name="ps_pv", bufs=1, space="PSUM") as ps_pv:
            qts = [keep.tile([128, S], BF16, name=f"qt{m}") for m in range(NM)]
            kts = [keep.tile([128, S], BF16, name=f"kt{m}") for m in range(NM)]
            vx = keep.tile([128, NKB, HLOC, DH + 1], BF16)
            ot = keep.tile([128, NM, S], F32)
            wq_sb = keep.tile([128, NDT, DLOC], BF16)
            wk_sb = keep.tile([128, NDT, DLOC], BF16)
            wv_sb = keep.tile([128, NDT, DLOC], BF16)
            xall = keep.tile([128, S // SC, NDT, SC], BF16)

            # DMAs spread across the three hwdge queues; the critical first
            # 2MB (x chunk 0 + Wk, gating the first K0 group) lands first.
            # Every transfer is contiguous per partition.
            nc.gpsimd.dma_start(xall[:, 0, 0:4], xT[:, 0, 0:4])
            nc.sync.dma_start(xall[:, 0, 4:8], xT[:, 0, 4:8])
            nc.scalar.dma_start(wk_sb[:, 0:4], Wk[:, 0:4])
            nc.gpsimd.dma_start(wk_sb[:, 4:8], Wk[:, 4:8])
            nc.sync.dma_start(wv_sb[:, 0:4], Wv[:, 0:4])
            nc.gpsimd.dma_start(wv_sb[:, 4:8], Wv[:, 4:8])
            nc.scalar.dma_start(wq_sb[:, 0:4], Wq[:, 0:4])
            nc.sync.dma_start(wq_sb[:, 4:8], Wq[:, 4:8])
            nc.gpsimd.dma_start(xall[:, 1], xT[:, 1])
            nc.gpsimd.dma_start(xall[:, 2], xT[:, 2])
            nc.gpsimd.dma_start(xall[:, 3], xT[:, 3])

            # vx column DH is the ones column (PV row DH = softmax
            # denominator; data rows stay at partition 0 — engines require
            # 32-aligned partition bases).
            ones_t = keep.tile([128, NKB, HLOC], BF16)
            nc.vector.memset(ones_t[:], 1.0)
            nc.vector.tensor_copy(vx[:, :, :, DH], ones_t[:])

            def v_group(sc, sb):
                ps = p1ps.tile([128, DLOC], F32, tag="p1")
                for dt_i in range(NDT):
                    nc.tensor.matmul(
                        ps[:], xall[:, sc, dt_i, sb * 128:(sb + 1) * 128],
                        wv_sb[:, dt_i, :],
                        start=(dt_i == 0), stop=(dt_i == NDT - 1))
                nc.vector.tensor_copy(
                    vx[:, sc * (SC // 128) + sb, :, 0:DH],
                    ps[:].rearrange("p (h d) -> p h d", h=HLOC))

            def kq_group(w_sb, dsts, m, sc):
                ps = p1ps.tile([128, SC], F32, tag="p1")
                ss = slice(sc * SC, (sc + 1) * SC)
                for dt_i in range(NDT):
                    nc.tensor.matmul(
                        ps[:], w_sb[:, dt_i, m * 128:(m + 1) * 128],
                        xall[:, sc, dt_i, :],
                        start=(dt_i == 0), stop=(dt_i == NDT - 1))
                nc.vector.tensor_copy(dsts[m][:, ss], ps[:])

            # ---- prefix: K for head-pair 0, Q0 chunk 0, first two V ------
            # (the remaining 14 V groups are threaded into q-block 0, whose
            # exps only depend on K/Q; pv(kb) needs V group kb at iter kb+1)
            with nc.named_scope("pre"):
                for sc in range(S // SC):
                    kq_group(wk_sb, kts, 0, sc)
                kq_group(wq_sb, qts, 0, 0)
                v_group(0, 0)
                v_group(0, 1)

            # ---- attention, with next head-pair's K/Q threaded in --------
            # Per hp the feed holds: this hp's remaining Q chunks (chunk q is
            # consumed by q-block q, and slot f fires at iteration 7f+6, well
            # before 16q), then the next hp's K (all chunks) and Q chunk 0 —
            # at most 8 groups, all done by iteration ~57 of the 64.
            with nc.named_scope("attn"):
                for hp in range(NM):
                    vfeed = []
                    if hp == 0:
                        for kbi in range(2, NKB):
                            vfeed.append(lambda s=kbi // 4, b=kbi % 4:
                                         v_group(s, b))
                    # This hp's K chunks 1-3 arrive as an "early feed" at
                    # iterations 0/5/9 (scores for k-block kb only need K
                    # chunk kb//4, i.e. by iterations 4/8/12) — that PE work
                    # moves out of the previous, PE-bound head-pair block.
                    efeed = []
                    if hp > 0:
                        for sc in range(1, S // SC):
                            efeed.append(lambda m=hp, s=sc:
                                         kq_group(wk_sb, kts, m, s))
                    feed = []
                    for sc in range(1, S // SC):
                        feed.append(lambda m=hp, s=sc:
                                    kq_group(wq_sb, qts, m, s))
                    if hp + 1 < NM:
                        feed.append(lambda m=hp + 1:
                                    kq_group(wk_sb, kts, m, 0))
                        feed.append(lambda m=hp + 1:
                                    kq_group(wq_sb, qts, m, 0))
                    for qc in range(S // QC):
                        qs = slice(qc * QC, (qc + 1) * QC)
                        pvs = [ps_pv.tile([DH + 1, QC], F32, tag=f"pv{h}",
                                          name=f"pv{h}") for h in range(2)]
                        exs = [None] * NKB

                        def emit_pv(kb):
                            for h in range(2):
                                nc.tensor.matmul(
                                    pvs[h][:], vx[:, kb, 2 * hp + h, :],
                                    exs[kb][:, h, :],
                                    start=(kb == 0), stop=(kb == NKB - 1),
                                    skip_group_check=True)

                        for kb in range(NKB):
                            ks = slice(kb * 128, (kb + 1) * 128)
                            spp = ps_s.tile([128, 2, QC], F32, tag="sc",
                                            name=f"sp{kb % 2}")
                            for h in range(2):
                                nc.tensor.matmul(
                                    spp[:, h, :],
                                    kts[hp][64 * h:64 * h + 64, ks],
                                    qts[hp][64 * h:64 * h + 64, qs],
                                    start=True, stop=True,
                                    tile_position=(64 * h, 0))
                            if kb > 0:
                                emit_pv(kb - 1)
                            ex = p2e.tile([128, 2, QC], BF16, tag="ex",
                                          name=f"ex{kb % 4}")
                            nc.scalar.activation(ex[:], spp[:], EXPF,
                                                 scale=1.0 / H)
                            exs[kb] = ex
                            it = qc * NKB + kb
                            if vfeed:
                                vfeed.pop(0)()
                            elif efeed and it in (0, 5, 9):
                                efeed.pop(0)()
                            elif feed and (
                                    (it >= 14 and (it - 14) % 8 == 0)
                                    if hp == 0 else
                                    (it >= 6 and (it - 6) % 9 == 0)):
                                feed.pop(0)()
                        emit_pv(NKB - 1)

                        # Copy pv out of PSUM right away (frees the bank for
                        # the next q-block), normalize from the SBUF copy.
                        for h in range(2):
                            dr = p2n.tile([1, QC], F32, tag="dr", name="dr")
                            nc.vector.tensor_copy(dr[:], pvs[h][DH:DH + 1, :])
                            pvc = p2n.tile([DH, QC], F32, tag=f"pvc{h}",
                                           name=f"pvc{h}")
                            nc.vector.tensor_copy(pvc[:], pvs[h][0:DH, :])
                            den = p2n.tile([1, QC], F32, tag="den", name="den")
                            nc.vector.reciprocal_approx_fast(den[:], dr[:])
                            bc = p2n.tile([DH, QC], F32, tag="bc", name="bc")
                            nc.gpsimd.partition_broadcast(bc[:], den[:])
                            nc.vector.tensor_mul(
                                ot[64 * h:64 * h + 64, hp, qs],
                                pvc[:], bc[:])
                            nc.gpsimd.dma_start(
                                out_t[64 * h:64 * h + 64, hp, qs],
                                ot[64 * h:64 * h + 64, hp, qs])

    nc.compile()
    return nc


def run(inputs, trace=False):
    x = np.asarray(inputs["encoder_input"], dtype=np.float32)
    Wq = np.asarray(inputs["Wq"], dtype=np.float32)
    Wk = np.asarray(inputs["Wk"], dtype=np.float32)
    Wv = np.asarray(inputs["Wv"], dtype=np.float32)

    nc = _build()
    bf = ml_dtypes.bfloat16

    def swz(a, inner):  # [R, C] -> [128, R//128, C], row o*128+p -> [p, o]
        return np.ascontiguousarray(
            a.reshape(-1, 128, inner).transpose(1, 0, 2)).astype(bf)

    in_maps = []
    for c in range(NCORES):
        b, g = c // 2, c % 2
        cols = slice(g * DLOC, (g + 1) * DLOC)
        xTc = np.ascontiguousarray(x[b].T)      # [D, S]
        in_maps.append({
            "xT": np.ascontiguousarray(
                xTc.reshape(NDT, 128, S // SC, SC)
                .transpose(1, 2, 0, 3)).astype(bf),
            "Wq": swz(np.ascontiguousarray(Wq[:, cols]), DLOC),
            "Wk": swz(np.ascontiguousarray(Wk[:, cols]), DLOC),
            "Wv": swz(np.ascontiguousarray(Wv[:, cols]), DLOC),
        })
    res = run_bass_kernel_spmd(nc, in_maps, core_ids=list(range(NCORES)),
                               trace=trace)
    out = np.empty((B, S, D), dtype=np.float32)
    for c in range(NCORES):
        b, g = c // 2, c % 2
        oT = res.results[c]["outT"].transpose(1, 0, 2).reshape(DLOC, S)
        out[b, :, g * DLOC:(g + 1) * DLOC] = oT.T
    return out, res


def kernel(**inputs):
    out, _ = run(inputs, trace=False)
    return out


# revision 45
# speedup vs baseline: 1.0016x; 1.0016x over previous
"""Multi-head self-attention (B=4, S=2048, D=1024, H=16) on 8 trn2 NeuronCores.

Sharding: core c -> batch b = c//2, head-group g = c%2 (8 heads, 512 of the
1024 output/QKV columns). Each core computes Q/K/V projections for its slice
and full attention for its 8 heads. Host does layout prep (x transpose + bf16
cast, W column slices) and the final gather/transpose - no collectives needed.

All matmuls in bf16 (psum accumulation f32): full PE rate, half the weight-load
time and DMA of f32r, and lower PE power draw (the f32r version tripped the HW
utilization throttle to ~54% duty).

Single fused pipeline per core (projections threaded into attention; the
ACT engine's exp stream is the binding resource at ~1.11us per k-block, so
everything else hides under it):
  prefix: K for head-pair 0, Q chunk 0, first two V groups. Inputs arrive
          via all three hwdge DMA queues, swizzled host-side so every
          transfer is contiguous per partition.
  per head-pair hp, per q-chunk(512): software-pipelined over 16 k-blocks:
      scoresT[k,q] psum[128,1024] <- KT-tile.T @ QT-chunk (2 heads, one bank
        each, tile_position rows 0/64 - the pair executes concurrently);
      one ACT exp over both banks -> ex bf16 [128,1024];
      pv[65,512] psum += Vx-tile.T @ ex-half (row 64 = denominator via a
        ones column in Vx), issued one k-block behind the scores;
      threaded into the iterations: the remaining V groups (during q-block
        0 of head-pair 0) and this/next head-pair's K/Q projection groups
        (one per ~6 iterations), so the PE fills its exp-stall slack.
    normalize: pv is copied out of PSUM immediately (frees the bank), then
      out = pv[0:64] * partition_broadcast(recip(pv[64])), per-head DMA out.
  output: outT[128,4,2048] f32 per core (partition-major); host transposes.
"""
import numpy as np
import ml_dtypes

import concourse.bacc as bacc
import concourse.mybir as mybir
import concourse.tile as tile
from concourse.bass_utils import run_bass_kernel_spmd

B, S, D, H = 4, 2048, 1024, 16
DH = D // H            # 64
NCORES = 8
HLOC = H // 2          # 8 heads per core
DLOC = HLOC * DH       # 512 output cols per core
NM = DLOC // 128       # 4 head-pair blocks
F32 = mybir.dt.float32
BF16 = mybir.dt.bfloat16
EXPF = mybir.ActivationFunctionType.Exp

SC = 512               # s-chunk for projections
QC = 512               # q-chunk in attention
NKB = S // 128         # 16 k-blocks
NDT = D // 128         # 8 contraction tiles for QKV


def _build():
    nc = bacc.Bacc("TRN2", target_bir_lowering=False, debug=False, num_devices=NCORES)
    # Host pre-swizzles everything partition-major so every DMA moves one
    # fully contiguous block per partition (32KB descriptors, not 1KB).
    xT = nc.dram_tensor("xT", [128, S // SC, NDT, SC], BF16,
                        kind="ExternalInput").ap()
    Wq = nc.dram_tensor("Wq", [128, NDT, DLOC], BF16, kind="ExternalInput").ap()
    Wk = nc.dram_tensor("Wk", [128, NDT, DLOC], BF16, kind="ExternalInput").ap()
    Wv = nc.dram_tensor("Wv", [128, NDT, DLOC], BF16, kind="ExternalInput").ap()
    out = nc.dram_tensor("outT", [128, NM, S], F32, kind="ExternalOutput").ap()

    out_t = out                                           # [128, 4, 2048]

    with tile.TileContext(nc) as tc:
        with tc.tile_pool(name="persist", bufs=1) as keep, \
             tc.tile_pool(name="p2e", bufs=4) as p2e, \
             tc.tile_pool(name="p2n", bufs=2) as p2n, \
             tc.tile_pool(name="p1ps", bufs=2, space="PSUM") as p1ps, \
             tc.tile_pool(name="ps_s", bufs=2, space="PSUM") as ps_s, \
             tc.tile_pool(name="ps_pv", bufs=1, space="PSUM") as ps_pv:
            qts = [keep.tile([128, S], BF16, name=f"qt{m}") for m in range(NM)]
            kts = [keep.tile([128, S], BF16, name=f"kt{m}") for m in range(NM)]
            vx = keep.tile([128, NKB, HLOC, DH + 1], BF16)
            ot = keep.tile([128, NM, S], F32)
            wq_sb = keep.tile([128, NDT, DLOC], BF16)
            wk_sb = keep.tile([128, NDT, DLOC], BF16)
            wv_sb = keep.tile([128, NDT, DLOC], BF16)
            xall = keep.tile([128, S // SC, NDT, SC], BF16)

            # DMAs spread across the three hwdge queues; the critical first
            # 2MB (x chunk 0 + Wk, gating the first K0 group) lands first.
            # Every transfer is contiguous per partition.
            nc.gpsimd.dma_start(xall[:, 0, 0:4], xT[:, 0, 0:4])
            nc.sync.dma_start(xall[:, 0, 4:8], xT[:, 0, 4:8])
            nc.scalar.dma_start(wk_sb[:, 0:4], Wk[:, 0:4])
            nc.gpsimd.dma_start(wk_sb[:, 4:8], Wk[:, 4:8])
            nc.sync.dma_start(wv_sb[:, 0:4], Wv[:, 0:4])
            nc.gpsimd.dma_start(wv_sb[:, 4:8], Wv[:, 4:8])
            nc.scalar.dma_start(wq_sb[:, 0:4], Wq[:, 0:4])
            nc.sync.dma_start(wq_sb[:, 4:8], Wq[:, 4:8])
            nc.gpsimd.dma_start(xall[:, 1], xT[:, 1])
            nc.gpsimd.dma_start(xall[:, 2], xT[:, 2])
            nc.gpsimd.dma_start(xall[:, 3], xT[:, 3])

            # vx column DH is the ones column (PV row DH = softmax
            # denominator; data rows stay at partition 0 — engines require
            # 32-aligned partition bases).
            ones_t = keep.tile([128, NKB, HLOC], BF16)
            nc.vector.memset(ones_t[:], 1.0)
            nc.vector.tensor_copy(vx[:, :, :, DH], ones_t[:])

            def v_group(sc, sb):
                ps = p1ps.tile([128, DLOC], F32, tag="p1")
                for dt_i in range(NDT):
                    nc.tensor.matmul(
                        ps[:], xall[:, sc, dt_i, sb * 128:(sb + 1) * 128],
                        wv_sb[:, dt_i, :],
                        start=(dt_i == 0), stop=(dt_i == NDT - 1))
                nc.vector.tensor_copy(
                    vx[:, sc * (SC // 128) + sb, :, 0:DH],
                    ps[:].rearrange("p (h d) -> p h d", h=HLOC))

            def kq_group(w_sb, dsts, m, sc):
                ps = p1ps.tile([128, SC], F32, tag="p1")
                ss = slice(sc * SC, (sc + 1) * SC)
                for dt_i in range(NDT):
                    nc.tensor.matmul(
                        ps[:], w_sb[:, dt_i, m * 128:(m + 1) * 128],
                        xall[:, sc, dt_i, :],
                        start=(dt_i == 0), stop=(dt_i == NDT - 1))
                nc.vector.tensor_copy(dsts[m][:, ss], ps[:])

            # ---- prefix: K for head-pair 0, Q0 chunk 0, first two V ------
            # (the remaining 14 V groups are threaded into q-block 0, whose
            # exps only depend on K/Q; pv(kb) needs V group kb at iter kb+1)
            with nc.named_scope("pre"):
                for sc in range(S // SC):
                    kq_group(wk_sb, kts, 0, sc)
                kq_group(wq_sb, qts, 0, 0)
                v_group(0, 0)
                v_group(0, 1)

            # ---- attention, with next head-pair's K/Q threaded in --------
            # Per hp the feed holds: this hp's remaining Q chunks (chunk q is
            # consumed by q-block q, and slot f fires at iteration 7f+6, well
            # before 16q), then the next hp's K (all chunks) and Q chunk 0 —
            # at most 8 groups, all done by iteration ~57 of the 64.
            with nc.named_scope("attn"):
                for hp in range(NM):
                    vfeed = []
                    if hp == 0:
                        for kbi in range(2, NKB):
                            vfeed.append(lambda s=kbi // 4, b=kbi % 4:
                                         v_group(s, b))
                    # This hp's K chunks 1-3 arrive as an "early feed" at
                    # iterations 0/5/9 (scores for k-block kb only need K
                    # chunk kb//4, i.e. by iterations 4/8/12) — that PE work
                    # moves out of the previous, PE-bound head-pair block.
                    efeed = []
                    if hp > 0:
                        for sc in range(1, S // SC):
                            efeed.append(lambda m=hp, s=sc:
                                         kq_group(wk_sb, kts, m, s))
                    feed = []
                    for sc in range(1, S // SC):
                        feed.append(lambda m=hp, s=sc:
                                    kq_group(wq_sb, qts, m, s))
                    if hp + 1 < NM:
                        feed.append(lambda m=hp + 1:
                                    kq_group(wk_sb, kts, m, 0))
                        feed.append(lambda m=hp + 1:
                                    kq_group(wq_sb, qts, m, 0))
                    for qc in range(S // QC):
                        qs = slice(qc * QC, (qc + 1) * QC)
                        pvs = [ps_pv.tile([DH + 1, QC], F32, tag=f"pv{h}",
                                          name=f"pv{h}") for h in range(2)]
                        exs = [None] * NKB

                        def emit_pv(kb):
                            for h in range(2):
                                nc.tensor.matmul(
                                    pvs[h][:], vx[:, kb, 2 * hp + h, :],
                                    exs[kb][:, h, :],
                                    start=(kb == 0), stop=(kb == NKB - 1),
                                    skip_group_check=True)

                        for kb in range(NKB):
                            ks = slice(kb * 128, (kb + 1) * 128)
                            spp = ps_s.tile([128, 2, QC], F32, tag="sc",
                                            name=f"sp{kb % 2}")
                            for h in range(2):
                                nc.tensor.matmul(
                                    spp[:, h, :],
                                    kts[hp][64 * h:64 * h + 64, ks],
                                    qts[hp][64 * h:64 * h + 64, qs],
                                    start=True, stop=True,
                                    tile_position=(64 * h, 0))
                            if kb > 0:
                                emit_pv(kb - 1)
                            ex = p2e.tile([128, 2, QC], BF16, tag="ex",
                                          name=f"ex{kb % 4}")
                            if hp == NM - 1 and qc == S // QC - 1 \
                                    and kb == NKB - 1:
                                # Kernel's very last exp: split per head so
                                # pv/normalization for head 0 (subtile deps)
                                # overlap head 1's exp, shortening the tail.
                                for h in range(2):
                                    nc.scalar.activation(
                                        ex[:, h, :], spp[:, h, :], EXPF,
                                        scale=1.0 / H)
                            else:
                                nc.scalar.activation(ex[:], spp[:], EXPF,
                                                     scale=1.0 / H)
                            exs[kb] = ex
                            it = qc * NKB + kb
                            if vfeed:
                                vfeed.pop(0)()
                            elif efeed and it in (0, 5, 9):
                                efeed.pop(0)()
                            elif feed and (
                                    (it >= 14 and (it - 14) % 8 == 0)
                                    if hp == 0 else
                                    (it >= 6 and (it - 6) % 9 == 0)):
                                feed.pop(0)()
                        emit_pv(NKB - 1)

                        # Copy pv out of PSUM right away (frees the bank for
                        # the next q-block), normalize from the SBUF copy.
                        for h in range(2):
                            dr = p2n.tile([1, QC], F32, tag="dr", name="dr")
                            nc.vector.tensor_copy(dr[:], pvs[h][DH:DH + 1, :])
                            pvc = p2n.tile([DH, QC], F32, tag=f"pvc{h}",
                                           name=f"pvc{h}")
                            nc.vector.tensor_copy(pvc[:], pvs[h][0:DH, :])
                            den = p2n.tile([1, QC], F32, tag="den", name="den")
                            nc.vector.reciprocal_approx_fast(den[:], dr[:])
                            bc = p2n.tile([DH, QC], F32, tag="bc", name="bc")
                            nc.gpsimd.partition_broadcast(bc[:], den[:])
                            nc.vector.tensor_mul(
                                ot[64 * h:64 * h + 64, hp, qs],
                                pvc[:], bc[:])
                            nc.gpsimd.dma_start(
                                out_t[64 * h:64 * h + 64, hp, qs],
                                ot[64 * h:64 * h + 64, hp, qs])

    nc.compile()
    return nc


def run(inputs, trace=False):
    x = np.asarray(inputs["encoder_input"], dtype=np.float32)
    Wq = np.asarray(inputs["Wq"], dtype=np.float32)
    Wk = np.asarray(inputs["Wk"], dtype=np.float32)
    Wv = np.asarray(inputs["Wv"], dtype=np.float32)

    nc = _build()
    bf = ml_dtypes.bfloat16

    def swz(a, inner):  # [R, C] -> [128, R//128, C], row o*128+p -> [p, o]
        return np.ascontiguousarray(
            a.reshape(-1, 128, inner).transpose(1, 0, 2)).astype(bf)

    in_maps = []
    for c in range(NCORES):
        b, g = c // 2, c % 2
        cols = slice(g * DLOC, (g + 1) * DLOC)
        xTc = np.ascontiguousarray(x[b].T)      # [D, S]
        in_maps.append({
            "xT": np.ascontiguousarray(
                xTc.reshape(NDT, 128, S // SC, SC)
                .transpose(1, 2, 0, 3)).astype(bf),
            "Wq": swz(np.ascontiguousarray(Wq[:, cols]), DLOC),
            "Wk": swz(np.ascontiguousarray(Wk[:, cols]), DLOC),
            "Wv": swz(np.ascontiguousarray(Wv[:, cols]), DLOC),
        })
    res = run_bass_kernel_spmd(nc, in_maps, core_ids=list(range(NCORES)),
                               trace=trace)
    out = np.empty((B, S, D), dtype=np.float32)
    for c in range(NCORES):
        b, g = c // 2, c % 2
        oT = res.results[c]["outT"].transpose(1, 0, 2).reshape(DLOC, S)
        out[b, :, g * DLOC:(g + 1) * DLOC] = oT.T
    return out, res


def kernel(**inputs):
    out, _ = run(inputs, trace=False)
    return out
